# revision 14
# baseline (speedup 1.0000x reference)
"""GCNConv on 8 trn2 NeuronCores.

out = segment_sum(edge_vals * (x @ weight)[edge_cols], edge_rows) + bias

Strategy:
  - Nodes (rows of x / out) sharded 8 ways: 12544 rows/core (padded 100352).
  - Phase 1: each core projects its x-shard: h_shard = x_shard @ W (f32 PE),
    stored as an fp16 gather table.
  - Phase 2: AllGather h shards -> full h table [100352, 128] fp16 in each
    core's DRAM.
  - Phase 3: each core aggregates its own destination rows. Edges are
    partitioned by destination owner on the host, sorted by (dest block,
    source quadrant), padded to 128-edge tiles. dma_gather fetches h[col]
    rows (<=1024 indices per call - ucode ring limit - spread over 4 SWDGE
    queues). The weighted segment-sum runs on the PE: per 128-edge tile a
    host-precomputed scatter matrix S[e, r] = val[e] * (row[e] == r) (fp16,
    streamed in bulk from DRAM) is matmul'd against the gathered messages,
    accumulating each 128-row output block in PSUM; bias is added on the
    PSUM->SBUF copy.
  - Source quadrants exist because dma_gather indices are int16 (<32768):
    the h table is viewed as 4 sub-tables of 25088 rows.

The per-(block, quadrant) tile counts are maxed across cores so all 8 cores
run one identical SPMD program (only input data differs).
"""

import numpy as np

import concourse.bacc as bacc
import concourse.bass as bass
import concourse.mybir as mybir
import concourse.tile as tile
from concourse.bass_utils import run_bass_kernel_spmd

N_NODES = 100000
N_EDGES = 1600000
IN_F = 256
OUT_F = 128
N_CORES = 8
P = 128

BLOCKS_PER_CORE = 98
ROWS_PER_CORE = BLOCKS_PER_CORE * P          # 12544
N_PAD = ROWS_PER_CORE * N_CORES              # 100352
N_QUAD = 4
QUAD_ROWS = N_PAD // N_QUAD                  # 25088 (< 32768, int16-safe)
QTR_ROWS = ROWS_PER_CORE // N_QUAD           # 3136 rows per rank quarter
SG_BLOCKS = 4                                # blocks per gather supergroup
N_SG = (BLOCKS_PER_CORE + SG_BLOCKS - 1) // SG_BLOCKS
CALL_TILES = 8                               # max 128-idx tiles per dma_gather
N_QUEUES = 4

_F32 = mybir.dt.float32
_F16 = mybir.dt.float16
_I16 = mybir.dt.int16

_cache = {}
TRACE = False
LAST_EXEC_NS = None


def _prep_host(x, edge_rows, edge_cols, edge_vals, weight, bias):
    """Shard + sort edges, build per-core padded meta arrays."""
    xT = np.zeros((IN_F, N_PAD), dtype=np.float16)
    xT[:, :N_NODES] = x.astype(np.float16).T
    xT = xT.reshape(2, P, N_PAD)
    w_f = np.ascontiguousarray(weight.astype(np.float16)).reshape(2, P, OUT_F)
    bias_b = np.broadcast_to(bias.astype(np.float32), (P, OUT_F)).copy()

    owner = edge_rows // ROWS_PER_CORE
    local_r = edge_rows - owner * ROWS_PER_CORE
    block = local_r >> 7
    rel = (local_r & 127).astype(np.int64)
    quad = edge_cols // QUAD_ROWS
    lcol = (edge_cols - quad * QUAD_ROWS).astype(np.int16)

    per_core = []
    counts = np.zeros((N_CORES, BLOCKS_PER_CORE, N_QUAD), dtype=np.int64)
    for c in range(N_CORES):
        m = owner == c
        cell_m = block[m] * N_QUAD + quad[m]
        order = np.argsort(cell_m, kind="stable")
        per_core.append(
            (cell_m[order], lcol[m][order], rel[m][order], edge_vals[m][order])
        )
        cnt = np.bincount(cell_m, minlength=BLOCKS_PER_CORE * N_QUAD)
        counts[c] = cnt.reshape(BLOCKS_PER_CORE, N_QUAD)

    # uniform-across-cores tile counts per (block, quadrant)
    T = (counts.max(axis=0) + P - 1) // P  # [98, 4]
    for b in range(BLOCKS_PER_CORE):
        if T[b].sum() == 0:
            T[b, 0] = 1
    T = T.astype(np.int64)

    # slot ordering: supergroup-major, quadrant, block, tile
    n_tiles = int(T.sum())
    slot_of = np.zeros((BLOCKS_PER_CORE, N_QUAD), dtype=np.int64)
    call_tiles = np.zeros((N_SG, N_QUAD), dtype=np.int64)  # tiles per (sg, q)
    s = 0
    for sg in range(N_SG):
        blo, bhi = sg * SG_BLOCKS, min((sg + 1) * SG_BLOCKS, BLOCKS_PER_CORE)
        for q in range(N_QUAD):
            for b in range(blo, bhi):
                slot_of[b, q] = s
                s += int(T[b, q])
            call_tiles[sg, q] = int(T[blo:bhi, q].sum())
    assert s == n_tiles

    # (sg, q) stream offsets in the idx16 buffer (in int16 columns = idxs/16)
    call_off16 = np.zeros((N_SG, N_QUAD), dtype=np.int64)
    off = 0
    for sg in range(N_SG):
        for q in range(N_QUAD):
            call_off16[sg, q] = off
            off += call_tiles[sg, q] * P // 16
    total_idx16 = off

    idx_hosts, smat_hosts = [], []
    for c in range(N_CORES):
        cell_c, lcol_c, rel_c, val_c = per_core[c]
        starts = np.zeros(BLOCKS_PER_CORE * N_QUAD + 1, dtype=np.int64)
        np.cumsum(counts[c].reshape(-1), out=starts[1:])
        # per-edge padded slot position: vectorized
        n_e = len(cell_c)
        # position of each edge within its cell
        pos_in_cell = np.arange(n_e) - starts[cell_c]
        slot0 = slot_of.reshape(-1)[cell_c]  # first slot of the edge's cell
        slot_e = slot0 + (pos_in_cell >> 7)
        part_e = (pos_in_cell & 127).astype(np.int64)

        idx_slots = np.zeros((n_tiles, P), dtype=np.int16)
        idx_slots[slot_e, part_e] = lcol_c
        # scatter matrices: smat[p, slot*128 + r] = val of edge (slot, p) if rel==r
        smat = np.zeros((P, n_tiles * P), dtype=np.float16)
        smat[part_e, slot_e * P + rel_c] = val_c.astype(np.float16)

        # idx16 layout: per (sg, q) stream, wrapped into 16 partitions,
        # replicated to 128 partitions
        idx_host = np.zeros((P, total_idx16), dtype=np.int16)
        for sg in range(N_SG):
            blo = sg * SG_BLOCKS
            for q in range(N_QUAD):
                nt = int(call_tiles[sg, q])
                if nt == 0:
                    continue
                s0 = int(slot_of[blo, q])
                stream = idx_slots[s0 : s0 + nt].reshape(-1)
                wrapped = stream.reshape(-1, 16).T  # [16, nt*8]
                o = int(call_off16[sg, q])
                idx_host[:, o : o + nt * 8] = np.tile(wrapped, (8, 1))
        idx_hosts.append(idx_host)
        smat_hosts.append(smat)

    meta = dict(
        T=T,
        slot_of=slot_of,
        call_tiles=call_tiles,
        call_off16=call_off16,
        n_tiles=n_tiles,
        total_idx16=total_idx16,
    )
    in_maps = []
    for c in range(N_CORES):
        in_maps.append(
            {
                "xT": np.ascontiguousarray(
                    xT[:, :, c * ROWS_PER_CORE : (c + 1) * ROWS_PER_CORE]
                ),
                "w": w_f,
                "biasb": bias_b,
                "idx16": idx_hosts[c],
                "smat": smat_hosts[c],
            }
        )
    return meta, in_maps


def _build_program(meta):
    T = meta["T"]
    slot_of = meta["slot_of"]
    call_tiles = meta["call_tiles"]
    call_off16 = meta["call_off16"]
    n_tiles = meta["n_tiles"]
    total_idx16 = meta["total_idx16"]

    nc = bacc.Bacc("TRN2", debug=False, num_devices=N_CORES, num_swdge_queues=N_QUEUES)

    xT_ap = nc.dram_tensor("xT", [2, P, ROWS_PER_CORE], _F16, kind="ExternalInput").ap()
    w_ap = nc.dram_tensor("w", [2, P, OUT_F], _F16, kind="ExternalInput").ap()
    biasb_ap = nc.dram_tensor("biasb", [P, OUT_F], _F32, kind="ExternalInput").ap()
    idx_ap = nc.dram_tensor("idx16", [P, total_idx16], _I16, kind="ExternalInput").ap()
    smat_ap = nc.dram_tensor("smat", [P, n_tiles * P], _F16, kind="ExternalInput").ap()
    out_ap = nc.dram_tensor("out", [ROWS_PER_CORE, OUT_F], _F32, kind="ExternalOutput").ap()

    h_shard = nc.dram_tensor("h_shard", [ROWS_PER_CORE, OUT_F], _F16).ap()
    h_full = nc.dram_tensor("h_full", [N_PAD, OUT_F], _F16, addr_space="Shared").ap()

    with tile.TileContext(nc) as tc:
        # ---------------- phase 1: h_shard = x_shard @ W ----------------
        with (
            tc.tile_pool(name="p1", bufs=1) as p1,
            tc.tile_pool(name="p1h", bufs=4) as p1h,
            tc.tile_pool(name="p1ps", bufs=4, space="PSUM") as p1ps,
        ):
            xt_sb = [
                p1.tile([P, ROWS_PER_CORE], _F16, tag=f"xt{k}", name=f"xt_sb{k}")
                for k in range(2)
            ]
            w_sb = [
                p1.tile([P, OUT_F], _F16, tag=f"w{k}", name=f"w_sb{k}")
                for k in range(2)
            ]
            for k in range(2):
                nc.sync.dma_start(out=xt_sb[k][:], in_=xT_ap[k])
                nc.sync.dma_start(out=w_sb[k][:], in_=w_ap[k])
            for nt in range(BLOCKS_PER_CORE):
                ph = p1ps.tile([P, OUT_F], _F32)
                for k in range(2):
                    nc.tensor.matmul(
                        ph[:],
                        lhsT=xt_sb[k][:, nt * P : (nt + 1) * P],
                        rhs=w_sb[k][:],
                        start=(k == 0),
                        stop=(k == 1),
                    )
                hb = p1h.tile([P, OUT_F], _F16)
                nc.vector.tensor_copy(out=hb[:], in_=ph[:])
                nc.sync.dma_start(out=h_shard[nt * P : (nt + 1) * P, :], in_=hb[:])

        # ---------------- phase 2: AllGather ----------------
        nc.gpsimd.collective_compute(
            "AllGather",
            mybir.AluOpType.bypass,
            replica_groups=[list(range(N_CORES))],
            ins=[h_shard[:]],
            outs=[h_full[:]],
        )

        # ---------------- phase 3: gather + scatter-matmul ----------------
        with (
            tc.tile_pool(name="meta", bufs=1) as pm,
            tc.tile_pool(name="gat", bufs=36) as pg,
            tc.tile_pool(name="spool", bufs=2) as psp,
            tc.tile_pool(name="outsb", bufs=4) as po,
            tc.tile_pool(name="psum3", bufs=4, space="PSUM") as pp,
        ):
            idx_sb = pm.tile([P, total_idx16], _I16)
            biasb_sb = pm.tile([P, OUT_F], _F32)
            nc.sync.dma_start(out=idx_sb[:], in_=idx_ap[:])
            nc.sync.dma_start(out=biasb_sb[:], in_=biasb_ap[:])

            call_no = 0
            for sg in range(N_SG):
                blo = sg * SG_BLOCKS
                bhi = min(blo + SG_BLOCKS, BLOCKS_PER_CORE)
                sg_slot0 = int(slot_of[blo, 0])
                sg_ntiles = int(T[blo:bhi].sum())
                # bulk-load this supergroup's scatter matrices
                s_sg = psp.tile([P, sg_ntiles * P], _F16, tag="smat", name=f"s_sg{sg}")
                nc.sync.dma_start(
                    out=s_sg[:],
                    in_=smat_ap[:, sg_slot0 * P : (sg_slot0 + sg_ntiles) * P],
                )
                g_calls = {}
                for q in range(N_QUAD):
                    nt_stream = int(call_tiles[sg, q])
                    o16 = int(call_off16[sg, q])
                    nchunks = (nt_stream + CALL_TILES - 1) // CALL_TILES
                    for k in range(nchunks):
                        ct = min(CALL_TILES, nt_stream - k * CALL_TILES)
                        g = pg.tile(
                            [P, ct, P], _F16, tag="gather", name=f"g_{sg}_{q}_{k}"
                        )
                        nc.gpsimd.dma_gather(
                            out_ap=g[:],
                            in_ap=h_full[q * QUAD_ROWS : (q + 1) * QUAD_ROWS, :],
                            idxs_ap=idx_sb[
                                :,
                                o16 + k * CALL_TILES * 8 : o16 + (k * CALL_TILES + ct) * 8,
                            ],
                            num_idxs=ct * P,
                            num_idxs_reg=ct * P,
                            elem_size=OUT_F,
                            queue_num=call_no % N_QUEUES,
                        )
                        call_no += 1
                        g_calls[(q, k)] = g
                for b in range(blo, bhi):
                    work = []  # (slot, gather_tile, pos_in_call)
                    for q in range(N_QUAD):
                        base = int(slot_of[b, q]) - int(slot_of[blo, q])
                        for t in range(int(T[b, q])):
                            sp = base + t
                            work.append(
                                (
                                    int(slot_of[b, q]) + t,
                                    g_calls[(q, sp // CALL_TILES)],
                                    sp % CALL_TILES,
                                )
                            )
                    ph3 = pp.tile([P, OUT_F], _F32, name="ph3")
                    for i, (slot, g, pos) in enumerate(work):
                        so = (slot - sg_slot0) * P
                        nc.tensor.matmul(
                            ph3[:],
                            lhsT=s_sg[:, so : so + P],
                            rhs=g[:, pos, :],
                            start=(i == 0),
                            stop=(i == len(work) - 1),
                        )
                    osb = po.tile([P, OUT_F], _F32, name="osb")
                    nc.vector.tensor_add(out=osb[:], in0=ph3[:], in1=biasb_sb[:])
                    nc.sync.dma_start(out=out_ap[b * P : (b + 1) * P, :], in_=osb[:])

    nc.compile()
    return nc


def kernel(x, edge_rows, edge_cols, edge_vals, weight, bias):
    x = np.asarray(x)
    edge_rows = np.asarray(edge_rows)
    edge_cols = np.asarray(edge_cols)
    edge_vals = np.asarray(edge_vals)
    weight = np.asarray(weight)
    bias = np.asarray(bias)

    meta, in_maps = _prep_host(x, edge_rows, edge_cols, edge_vals, weight, bias)

    key = meta["T"].tobytes()
    if key not in _cache:
        _cache[key] = _build_program(meta)
    nc = _cache[key]

    res = run_bass_kernel_spmd(nc, in_maps, list(range(N_CORES)), trace=TRACE)
    global LAST_EXEC_NS
    LAST_EXEC_NS = res.exec_time_ns
    out = np.concatenate([res.results[c]["out"] for c in range(N_CORES)], axis=0)
    return out[:N_NODES].astype(np.float32)


# revision 16
# speedup vs baseline: 1.3341x; 1.3341x over previous
"""GCNConv on 8 trn2 NeuronCores.

out = segment_sum(edge_vals * (x @ weight)[edge_cols], edge_rows) + bias

Strategy:
  - Nodes (rows of x / out) sharded 8 ways: 12544 rows/core (padded 100352).
  - Phase 1: each core projects its x-shard: h_shard = x_shard @ W (f32 PE),
    stored as an fp16 gather table.
  - Phase 2: AllGather h shards -> full h table [100352, 128] fp16 in each
    core's DRAM.
  - Phase 3: each core aggregates its own destination rows. Edges are
    partitioned by destination owner on the host, sorted by (dest block,
    source quadrant), padded to 128-edge tiles. dma_gather fetches h[col]
    rows (<=1024 indices per call - ucode ring limit - spread over 4 SWDGE
    queues). The weighted segment-sum runs on the PE: per 128-edge tile a
    host-precomputed scatter matrix S[e, r] = val[e] * (row[e] == r) (fp16,
    streamed in bulk from DRAM) is matmul'd against the gathered messages,
    accumulating each 128-row output block in PSUM; bias is added on the
    PSUM->SBUF copy.
  - Source quadrants exist because dma_gather indices are int16 (<32768):
    the h table is viewed as 4 sub-tables of 25088 rows.

The per-(block, quadrant) tile counts are maxed across cores so all 8 cores
run one identical SPMD program (only input data differs).
"""

import numpy as np

import concourse.bacc as bacc
import concourse.bass as bass
import concourse.mybir as mybir
import concourse.tile as tile
from concourse.bass_utils import run_bass_kernel_spmd

N_NODES = 100000
N_EDGES = 1600000
IN_F = 256
OUT_F = 128
N_CORES = 8
P = 128

BLOCKS_PER_CORE = 98                         # phase-1 projection tiling
ROWS_PER_CORE = BLOCKS_PER_CORE * P          # 12544 (padded shard rows)
RREAL = N_NODES // N_CORES                   # 12500 real rows per core
AGG_BLOCKS = 112                             # aggregation blocks (span <=112 rows)
N_PAD = ROWS_PER_CORE * N_CORES              # 100352
N_QUAD = 4
QUAD_ROWS = N_PAD // N_QUAD                  # 25088 (< 32768, int16-safe)
QTR_ROWS = ROWS_PER_CORE // N_QUAD           # 3136 rows per rank quarter
SG_BLOCKS = 4                                # blocks per gather supergroup
N_SG = (AGG_BLOCKS + SG_BLOCKS - 1) // SG_BLOCKS
CALL_TILES = 8                               # max 128-idx tiles per dma_gather
N_QUEUES = 4

_F32 = mybir.dt.float32
_F16 = mybir.dt.float16
_I16 = mybir.dt.int16

_cache = {}
TRACE = False
LAST_EXEC_NS = None


def _prep_host(x, edge_rows, edge_cols, edge_vals, weight, bias):
    """Shard + sort edges, build per-core padded meta arrays."""
    xT = np.zeros((IN_F, N_PAD), dtype=np.float16)
    xt_f16 = x.astype(np.float16).T
    for c in range(N_CORES):
        xT[:, c * ROWS_PER_CORE : c * ROWS_PER_CORE + RREAL] = xt_f16[
            :, c * RREAL : (c + 1) * RREAL
        ]
    xT = xT.reshape(2, P, N_PAD)
    w_f = np.ascontiguousarray(weight.astype(np.float16)).reshape(2, P, OUT_F)
    bias_b = np.broadcast_to(bias.astype(np.float32), (P, OUT_F)).copy()

    st = (np.arange(AGG_BLOCKS + 1, dtype=np.int64) * RREAL + AGG_BLOCKS - 1) // AGG_BLOCKS
    owner = edge_rows // RREAL
    local_r = (edge_rows - owner * RREAL).astype(np.int64)
    block = (local_r * AGG_BLOCKS) // RREAL
    rel = local_r - st[block]
    c_owner = edge_cols // RREAL
    c_table = c_owner * ROWS_PER_CORE + (edge_cols - c_owner * RREAL)
    quad = c_table // QUAD_ROWS
    lcol = (c_table - quad * QUAD_ROWS).astype(np.int16)

    per_core = []
    counts = np.zeros((N_CORES, AGG_BLOCKS, N_QUAD), dtype=np.int64)
    for c in range(N_CORES):
        m = owner == c
        cell_m = block[m] * N_QUAD + quad[m]
        order = np.argsort(cell_m, kind="stable")
        per_core.append(
            (cell_m[order], lcol[m][order], rel[m][order], edge_vals[m][order])
        )
        cnt = np.bincount(cell_m, minlength=AGG_BLOCKS * N_QUAD)
        counts[c] = cnt.reshape(AGG_BLOCKS, N_QUAD)

    # uniform-across-cores tile counts per (block, quadrant)
    T = (counts.max(axis=0) + P - 1) // P  # [AGG_BLOCKS, 4]
    for b in range(AGG_BLOCKS):
        if T[b].sum() == 0:
            T[b, 0] = 1
    T = T.astype(np.int64)

    # slot ordering: supergroup-major, quadrant, block, tile
    n_tiles = int(T.sum())
    slot_of = np.zeros((AGG_BLOCKS, N_QUAD), dtype=np.int64)
    call_tiles = np.zeros((N_SG, N_QUAD), dtype=np.int64)  # tiles per (sg, q)
    s = 0
    for sg in range(N_SG):
        blo, bhi = sg * SG_BLOCKS, min((sg + 1) * SG_BLOCKS, AGG_BLOCKS)
        for q in range(N_QUAD):
            for b in range(blo, bhi):
                slot_of[b, q] = s
                s += int(T[b, q])
            call_tiles[sg, q] = int(T[blo:bhi, q].sum())
    assert s == n_tiles

    # (sg, q) stream offsets in the idx16 buffer (in int16 columns = idxs/16)
    call_off16 = np.zeros((N_SG, N_QUAD), dtype=np.int64)
    off = 0
    for sg in range(N_SG):
        for q in range(N_QUAD):
            call_off16[sg, q] = off
            off += call_tiles[sg, q] * P // 16
    total_idx16 = off

    idx_hosts, smat_hosts = [], []
    for c in range(N_CORES):
        cell_c, lcol_c, rel_c, val_c = per_core[c]
        starts = np.zeros(AGG_BLOCKS * N_QUAD + 1, dtype=np.int64)
        np.cumsum(counts[c].reshape(-1), out=starts[1:])
        # per-edge padded slot position: vectorized
        n_e = len(cell_c)
        # position of each edge within its cell
        pos_in_cell = np.arange(n_e) - starts[cell_c]
        slot0 = slot_of.reshape(-1)[cell_c]  # first slot of the edge's cell
        slot_e = slot0 + (pos_in_cell >> 7)
        part_e = (pos_in_cell & 127).astype(np.int64)

        idx_slots = np.zeros((n_tiles, P), dtype=np.int16)
        idx_slots[slot_e, part_e] = lcol_c
        # scatter matrices: smat[p, slot*128 + r] = val of edge (slot, p) if rel==r
        smat = np.zeros((P, n_tiles * P), dtype=np.float16)
        smat[part_e, slot_e * P + rel_c] = val_c.astype(np.float16)

        # idx16 layout: per (sg, q) stream, wrapped into 16 partitions,
        # replicated to 128 partitions
        idx_host = np.zeros((P, total_idx16), dtype=np.int16)
        for sg in range(N_SG):
            blo = sg * SG_BLOCKS
            for q in range(N_QUAD):
                nt = int(call_tiles[sg, q])
                if nt == 0:
                    continue
                s0 = int(slot_of[blo, q])
                stream = idx_slots[s0 : s0 + nt].reshape(-1)
                wrapped = stream.reshape(-1, 16).T  # [16, nt*8]
                o = int(call_off16[sg, q])
                idx_host[:, o : o + nt * 8] = np.tile(wrapped, (8, 1))
        idx_hosts.append(idx_host)
        smat_hosts.append(smat)

    meta = dict(
        st=st,
        T=T,
        slot_of=slot_of,
        call_tiles=call_tiles,
        call_off16=call_off16,
        n_tiles=n_tiles,
        total_idx16=total_idx16,
    )
    in_maps = []
    for c in range(N_CORES):
        in_maps.append(
            {
                "xT": np.ascontiguousarray(
                    xT[:, :, c * ROWS_PER_CORE : (c + 1) * ROWS_PER_CORE]
                ),
                "w": w_f,
                "biasb": bias_b,
                "idx16": idx_hosts[c],
                "smat": smat_hosts[c],
            }
        )
    return meta, in_maps


def _build_program(meta):
    st = meta["st"]
    T = meta["T"]
    slot_of = meta["slot_of"]
    call_tiles = meta["call_tiles"]
    call_off16 = meta["call_off16"]
    n_tiles = meta["n_tiles"]
    total_idx16 = meta["total_idx16"]

    nc = bacc.Bacc("TRN2", debug=False, num_devices=N_CORES, num_swdge_queues=N_QUEUES)

    xT_ap = nc.dram_tensor("xT", [2, P, ROWS_PER_CORE], _F16, kind="ExternalInput").ap()
    w_ap = nc.dram_tensor("w", [2, P, OUT_F], _F16, kind="ExternalInput").ap()
    biasb_ap = nc.dram_tensor("biasb", [P, OUT_F], _F32, kind="ExternalInput").ap()
    idx_ap = nc.dram_tensor("idx16", [P, total_idx16], _I16, kind="ExternalInput").ap()
    smat_ap = nc.dram_tensor("smat", [P, n_tiles * P], _F16, kind="ExternalInput").ap()
    out_ap = nc.dram_tensor("out", [RREAL, OUT_F], _F32, kind="ExternalOutput").ap()

    h_shard = nc.dram_tensor("h_shard", [ROWS_PER_CORE, OUT_F], _F16).ap()
    h_full = nc.dram_tensor("h_full", [N_PAD, OUT_F], _F16, addr_space="Shared").ap()

    with tile.TileContext(nc) as tc:
        # ---------------- phase 1: h_shard = x_shard @ W ----------------
        with (
            tc.tile_pool(name="p1", bufs=1) as p1,
            tc.tile_pool(name="p1h", bufs=4) as p1h,
            tc.tile_pool(name="p1ps", bufs=4, space="PSUM") as p1ps,
        ):
            xt_sb = [
                p1.tile([P, ROWS_PER_CORE], _F16, tag=f"xt{k}", name=f"xt_sb{k}")
                for k in range(2)
            ]
            w_sb = [
                p1.tile([P, OUT_F], _F16, tag=f"w{k}", name=f"w_sb{k}")
                for k in range(2)
            ]
            for k in range(2):
                nc.sync.dma_start(out=xt_sb[k][:], in_=xT_ap[k])
                nc.sync.dma_start(out=w_sb[k][:], in_=w_ap[k])
            for nt in range(BLOCKS_PER_CORE):
                ph = p1ps.tile([P, OUT_F], _F32)
                for k in range(2):
                    nc.tensor.matmul(
                        ph[:],
                        lhsT=xt_sb[k][:, nt * P : (nt + 1) * P],
                        rhs=w_sb[k][:],
                        start=(k == 0),
                        stop=(k == 1),
                    )
                hb = p1h.tile([P, OUT_F], _F16)
                nc.vector.tensor_copy(out=hb[:], in_=ph[:])
                nc.sync.dma_start(out=h_shard[nt * P : (nt + 1) * P, :], in_=hb[:])

        # ---------------- phase 2: AllGather ----------------
        nc.gpsimd.collective_compute(
            "AllGather",
            mybir.AluOpType.bypass,
            replica_groups=[list(range(N_CORES))],
            ins=[h_shard[:]],
            outs=[h_full[:]],
        )

        # ---------------- phase 3: gather + scatter-matmul ----------------
        with (
            tc.tile_pool(name="meta", bufs=1) as pm,
            tc.tile_pool(name="gat", bufs=24) as pg,
            tc.tile_pool(name="spool", bufs=2) as psp,
            tc.tile_pool(name="outsb", bufs=4) as po,
            tc.tile_pool(name="psum3", bufs=4, space="PSUM") as pp,
        ):
            idx_sb = pm.tile([P, total_idx16], _I16)
            biasb_sb = pm.tile([P, OUT_F], _F32)
            nc.sync.dma_start(out=idx_sb[:], in_=idx_ap[:])
            nc.sync.dma_start(out=biasb_sb[:], in_=biasb_ap[:])

            call_no = 0
            for sg in range(N_SG):
                blo = sg * SG_BLOCKS
                bhi = min(blo + SG_BLOCKS, AGG_BLOCKS)
                sg_slot0 = int(slot_of[blo, 0])
                sg_ntiles = int(T[blo:bhi].sum())
                # bulk-load this supergroup's scatter matrices
                s_sg = psp.tile([P, sg_ntiles * P], _F16, tag="smat", name=f"s_sg{sg}")
                nc.sync.dma_start(
                    out=s_sg[:],
                    in_=smat_ap[:, sg_slot0 * P : (sg_slot0 + sg_ntiles) * P],
                )
                g_calls = {}
                for q in range(N_QUAD):
                    nt_stream = int(call_tiles[sg, q])
                    o16 = int(call_off16[sg, q])
                    nchunks = (nt_stream + CALL_TILES - 1) // CALL_TILES
                    for k in range(nchunks):
                        ct = min(CALL_TILES, nt_stream - k * CALL_TILES)
                        g = pg.tile(
                            [P, ct, P], _F16, tag="gather", name=f"g_{sg}_{q}_{k}"
                        )
                        nc.gpsimd.dma_gather(
                            out_ap=g[:],
                            in_ap=h_full[q * QUAD_ROWS : (q + 1) * QUAD_ROWS, :],
                            idxs_ap=idx_sb[
                                :,
                                o16 + k * CALL_TILES * 8 : o16 + (k * CALL_TILES + ct) * 8,
                            ],
                            num_idxs=ct * P,
                            num_idxs_reg=ct * P,
                            elem_size=OUT_F,
                            queue_num=call_no % N_QUEUES,
                        )
                        call_no += 1
                        g_calls[(q, k)] = g
                for b in range(blo, bhi):
                    work = []  # (slot, gather_tile, pos_in_call)
                    for q in range(N_QUAD):
                        base = int(slot_of[b, q]) - int(slot_of[blo, q])
                        for t in range(int(T[b, q])):
                            sp = base + t
                            work.append(
                                (
                                    int(slot_of[b, q]) + t,
                                    g_calls[(q, sp // CALL_TILES)],
                                    sp % CALL_TILES,
                                )
                            )
                    ph3 = pp.tile([P, OUT_F], _F32, name="ph3")
                    for i, (slot, g, pos) in enumerate(work):
                        so = (slot - sg_slot0) * P
                        nc.tensor.matmul(
                            ph3[:],
                            lhsT=s_sg[:, so : so + P],
                            rhs=g[:, pos, :],
                            start=(i == 0),
                            stop=(i == len(work) - 1),
                        )
                    osb = po.tile([P, OUT_F], _F32, name="osb")
                    nc.vector.tensor_add(out=osb[:], in0=ph3[:], in1=biasb_sb[:])
                    cnt = int(st[b + 1] - st[b])
                    nc.sync.dma_start(
                        out=out_ap[int(st[b]) : int(st[b + 1]), :], in_=osb[:cnt, :]
                    )

    nc.compile()
    return nc


def kernel(x, edge_rows, edge_cols, edge_vals, weight, bias):
    x = np.asarray(x)
    edge_rows = np.asarray(edge_rows)
    edge_cols = np.asarray(edge_cols)
    edge_vals = np.asarray(edge_vals)
    weight = np.asarray(weight)
    bias = np.asarray(bias)

    meta, in_maps = _prep_host(x, edge_rows, edge_cols, edge_vals, weight, bias)

    key = meta["T"].tobytes()
    if key not in _cache:
        _cache[key] = _build_program(meta)
    nc = _cache[key]

    res = run_bass_kernel_spmd(nc, in_maps, list(range(N_CORES)), trace=TRACE)
    global LAST_EXEC_NS
    LAST_EXEC_NS = res.exec_time_ns
    out = np.concatenate([res.results[c]["out"] for c in range(N_CORES)], axis=0)
    return out.astype(np.float32)
